# revision 27
# baseline (speedup 1.0000x reference)
"""Causal multi-head attention on 8 trn2 NeuronCores.

Problem: B=4, S=2048, D=2048, H=16 heads, head_dim=128, causal softmax,
torch-style Linear projections (W stored [in, out]).

Sharding: core c handles batch b = c//2 and head-group g = c%2
(8 heads = 1024 output columns of Wq/Wk/Wv, 1024 rows of Wo).
Each core produces a partial output [S, D]; host sums the two
head-group partials per batch and adds bo.

All matmul operands fp16 (PSUM accumulation stays fp32): fp16 streams
at 1 row/cycle on the PE vs fp32r's ~1.25, and halves DMA/SBUF bytes.

Per-core device pipeline:
  Phase A: from xT (host-pretransposed [D, S]) compute
           Q^T, K^T [1024, S] (spilled to DRAM scratch) and V [S, 1024]
           (kept SBUF-resident: 32KB/partition in fp16).
  Phase B: per head h, per 512-wide q-chunk: k-tile PAIRS:
           scores^T strips [128 k, 512 q] = K_h Q_h^T into a 2-bank
           psum pair, one exp per pair (psum -> fp16 SBUF, no mask),
           causal upper-triangle + garbage zeroed by one gpsimd
           affine_select per diagonal pair (exact: zeros contribute
           nothing downstream), ctx^T accumulation C^T = V_h^T @ P^T,
           denominator accumulated on DVE in fp16 (one ones-matmul per
           q-chunk instead of one per tile), normalize into
           SBUF-resident C^T (32KB/partition).
  Phase C: out_partial = C @ Wo_slice straight from SBUF ct tiles.
"""

import numpy as np

import concourse.bass as bass
import concourse.mybir as mybir
import concourse.tile as tile
from concourse import bacc
from concourse.bass_utils import run_bass_kernel_spmd

B = 4
S = 2048
D = 2048
H = 16
DH = 128
HPC = 8          # heads per core
DHG = HPC * DH   # 1024: head-group width per core
KT = D // 128    # 16 k-tiles over the model dim
ST = S // 128    # 16 s-tiles
QC = S // 512    # 4 q-chunks
SCALE = 1.0 / np.sqrt(DH)

F32 = mybir.dt.float32
F16 = mybir.dt.float16


def _build_nc():
    nc = bacc.Bacc(None, target_bir_lowering=False)

    xT = nc.declare_dram_parameter("xT", [D, S], F16, isOutput=False)
    # wq/wk host-pregathered to [HPC*128, KT*128]: row t*128+p, col n*128+m
    # = Wq[n*128+p, t*128+m] so each head-tile's weights DMA contiguously
    wq = nc.declare_dram_parameter("wq", [DHG, D], F16, isOutput=False)
    wk = nc.declare_dram_parameter("wk", [DHG, D], F16, isOutput=False)
    wv = nc.declare_dram_parameter("wv", [D, DHG], F16, isOutput=False)
    wo = nc.declare_dram_parameter("wo", [DHG, D], F16, isOutput=False)
    bqT = nc.declare_dram_parameter("bqT", [128, HPC], F32, isOutput=False)
    bkT = nc.declare_dram_parameter("bkT", [128, HPC], F32, isOutput=False)
    bvb = nc.declare_dram_parameter("bvb", [128, DHG], F32, isOutput=False)
    out = nc.declare_dram_parameter("out", [S, D], F32, isOutput=True)

    with tile.TileContext(nc) as tc:
        _emit(nc, tc, xT, wq, wk, wv, wo, bqT, bkT, bvb, out)
    nc.compile()
    return nc


def _emit(nc, tc, xT, wq, wk, wv, wo, bqT, bkT, bvb, out):
    with (
        tc.tile_pool(name="const", bufs=1) as const,
        tc.tile_pool(name="dram", bufs=1, space="DRAM") as dram,
        tc.tile_pool(name="qkpre", bufs=2) as qkpre,
        tc.tile_pool(name="vfull", bufs=1) as vfull,
    ):
        qt_d = dram.tile([DHG, S], F16)
        kt_d = dram.tile([DHG, S], F16)

        bq_sb = const.tile([128, HPC], F32)
        bk_sb = const.tile([128, HPC], F32)
        bv_sb = const.tile([128, DHG], F32)
        ones_f32 = const.tile([128, 128], F32)
        nc.vector.memset(ones_f32, 1.0)
        ones128 = const.tile([128, 128], F16)
        nc.vector.tensor_copy(out=ones128, in_=ones_f32)

        # V [S, DHG] stays in SBUF for the whole kernel (ct_full and wo_sb
        # allocate lazily at phase B so their space is free during phase A)
        v_full = vfull.tile([128, ST, DHG], F16)

        head0 = {}  # head-0 q/k tiles, prefetched during phase A

        # ---------------- Phase A: projections -------------------------------
        wv_r = wv.rearrange("(n p) m -> p n m", p=128)

        with (
            tc.tile_pool(name="xts", bufs=6) as xtp,
            tc.tile_pool(name="wqk", bufs=2) as wqk,
            tc.tile_pool(name="wvp", bufs=1) as wvp,
            tc.tile_pool(name="apsum", bufs=8, space="PSUM") as aps,
            tc.tile_pool(name="astage", bufs=4) as ast,
        ):
            wv_full = wvp.tile([128, KT, DHG], F16)

            for sh in range(2):
                s0 = sh * (S // 2)
                # weight tiles prefetched (depth 2) ahead of the bulk xT DMAs
                seq = [(w, b, dst, t)
                       for w, b, dst in ((wq, bq_sb, qt_d), (wk, bk_sb, kt_d))
                       for t in range(HPC)]
                w_tiles = {}

                def w_prefetch(i):
                    if i < len(seq):
                        w, _, _, t = seq[i]
                        w_sb = wqk.tile([128, KT, 128], F16, tag="wqk",
                                        name=f"w_sb{i % 2}")
                        nc.sync.dma_start(
                            out=w_sb,
                            in_=w[t * 128 : (t + 1) * 128, :]
                            .rearrange("p (n m) -> p n m", m=128),
                        )
                        w_tiles[i] = w_sb

                w_prefetch(0)
                if sh == 0:
                    # bias loads queue behind the critical first weight tile
                    nc.sync.dma_start(out=bq_sb, in_=bqT[:, :])
                    nc.sync.dma_start(out=bk_sb, in_=bkT[:, :])

                # xT half as four sub-tiles of 4 k-tiles each, DMA'd per
                # k-tile: tile-granular dependency tracking then lets the
                # first matmuls start after only a few k-tiles have landed.
                # Loads go on the gpsimd queue so the w_sb prefetch chain on
                # the sync queue is never stuck behind them.
                xt_sub = [
                    xtp.tile([128, 4, S // 2], F16, tag="xts",
                             name=f"xt{sh}{q}")
                    for q in range(4)
                ]

                def xt_blk(kd):
                    return xt_sub[kd // 4][:, kd % 4, :]

                # alternate issue queues: 16 issues at ~640ns each on one
                # queue would feed tiles slower than the first QK sweep
                # consumes them
                for kd in range(KT):
                    eng = nc.gpsimd if kd % 2 == 0 else nc.scalar
                    eng.dma_start(
                        out=xt_blk(kd),
                        in_=xT[kd * 128 : (kd + 1) * 128, s0 : s0 + S // 2],
                    )

                if sh == 0:
                    # bvb is only needed at V time
                    nc.scalar.dma_start(out=bv_sb, in_=bvb[:, :])

                # Q^T and K^T: psum[dh 128, s 512] = sum_kd Wblk^T @ xTblk
                for i, (w, b_sb, dst, t) in enumerate(seq):
                    w_sb = w_tiles.pop(i)
                    w_prefetch(i + 1)
                    for sc in range(2):
                        psum = aps.tile([128, 512], F32, tag="apsum", name="qk_ps")
                        for kd in range(KT):
                            nc.tensor.matmul(
                                psum,
                                w_sb[:, kd, :],
                                xt_blk(kd)[:, sc * 512 : (sc + 1) * 512],
                                start=(kd == 0),
                                stop=(kd == KT - 1),
                            )
                        stg = ast.tile([128, 512], F16, tag="astage")
                        nc.vector.tensor_scalar_add(
                            out=stg, in0=psum, scalar1=b_sb[:, t : t + 1]
                        )
                        nc.gpsimd.dma_start(
                            out=dst[
                                t * 128 : (t + 1) * 128,
                                s0 + sc * 512 : s0 + (sc + 1) * 512,
                            ],
                            in_=stg,
                        )
                    if sh == 0 and i == 3:
                        # 4MB wv load on the gpsimd queue, emitted behind the
                        # first QK spills: the spill dma_starts wait on their
                        # compute, so wv's ~15us descriptor flood hits the
                        # rings only after the latency-critical xT tiles have
                        # all landed. V first needs wv ~70us later.
                        nc.gpsimd.dma_start(out=wv_full, in_=wv_r[:, :, :])

                if sh == 1:
                    qt0 = qkpre.tile([128, S], F16, tag="qt", name="qt0")
                    nc.sync.dma_start(out=qt0, in_=qt_d[0:128, :])
                    kt0 = qkpre.tile([128, S], F16, tag="kt", name="kt0")
                    nc.sync.dma_start(out=kt0, in_=kt_d[0:128, :])
                    head0["qt"] = qt0
                    head0["kt"] = kt0

                # V: psum[s 128, dh 512] = sum_kd xTblk^T @ Wvblk; bias-added
                # result written straight into the SBUF-resident v_full.
                # si-groups of 4 so each group's DVE drain overlaps the next
                # group's matmul sweep (and the tail drain is short).
                for t2 in range(2):
                    for sg in range(2):
                        psums = [
                            aps.tile([128, 512], F32, tag="apsum",
                                     name=f"vps{si}")
                            for si in range(4)
                        ]
                        for kd in range(KT):
                            for si in range(4):
                                nc.tensor.matmul(
                                    psums[si],
                                    xt_blk(kd)[:, (sg * 4 + si) * 128 :
                                               (sg * 4 + si + 1) * 128],
                                    wv_full[:, kd, t2 * 512 : (t2 + 1) * 512],
                                    start=(kd == 0),
                                    stop=(kd == KT - 1),
                                )
                        for si in range(4):
                            nc.vector.tensor_tensor(
                                out=v_full[:, sh * 8 + sg * 4 + si,
                                           t2 * 512 : (t2 + 1) * 512],
                                in0=psums[si],
                                in1=bv_sb[:, t2 * 512 : (t2 + 1) * 512],
                                op=mybir.AluOpType.add,
                            )

        # ---------------- Phase B: per-head attention ------------------------
        wo_r = wo.rearrange("(n p) m -> p n m", p=128)
        # ct_full / wo_sb pools open only now (span B and C) so their 64KB
        # per partition is free during phase A
        ctfull_cm = tc.tile_pool(name="ctfull", bufs=1)
        ctfull = ctfull_cm.__enter__()
        wop_cm = tc.tile_pool(name="wop", bufs=1)
        wop = wop_cm.__enter__()
        with (
            tc.tile_pool(name="ptile", bufs=4) as ppool,
            tc.tile_pool(name="pacc", bufs=2) as paccp,
            tc.tile_pool(name="rcp", bufs=2) as rcpool,
            tc.tile_pool(name="pscore", bufs=2, space="PSUM") as pscore,
            tc.tile_pool(name="pctx", bufs=2, space="PSUM") as pctx,
            tc.tile_pool(name="psum2", bufs=2, space="PSUM") as psums,
        ):
            ct_full = ctfull.tile([128, HPC, S], F16)
            wo_sb = wop.tile([128, HPC, D], F16)

            # Flat (head, qc, pair) pipeline.  Scores run one pair ahead of
            # ctx, crossing qc/head boundaries so the PE never drains at a
            # boundary; per-qc finalization (denominator matmul + recip +
            # normalize) is deferred two pair-units so its inputs are ready.
            units = []
            for h in range(HPC):
                for qc in range(QC):
                    prs = [(4 * qc, 4 * qc + 1), (4 * qc + 2, 4 * qc + 3)]
                    prs += [(t, t + 1) for t in range(0, 4 * qc, 2)]
                    for pi, pair in enumerate(prs):
                        units.append({
                            "h": h, "qc": qc, "pair": pair,
                            "first": pi == 0, "last": pi == len(prs) - 1,
                        })

            qk_tiles = {0: (head0["qt"], head0["kt"])}

            def load_head(h):
                if h in qk_tiles or h >= HPC:
                    return
                qt_sb = qkpre.tile([128, S], F16, tag="qt", name="qt_sb")
                nc.sync.dma_start(out=qt_sb, in_=qt_d[h * 128 : (h + 1) * 128, :])
                kt_sb = qkpre.tile([128, S], F16, tag="kt", name="kt_sb")
                nc.sync.dma_start(out=kt_sb, in_=kt_d[h * 128 : (h + 1) * 128, :])
                qk_tiles[h] = (qt_sb, kt_sb)

            state = {}      # (h, qc) -> dict with psum_c, psum_s, p_acc
            finalizes = []  # (due_iteration, h, qc)

            def scores(u):
                h, qc = u["h"], u["qc"]
                qt_sb, kt_sb = qk_tiles[h]
                pp = pscore.tile([128, 1024], F32, tag="pp")
                offs = []
                for half, kt_i in enumerate(u["pair"]):
                    j = kt_i - 4 * qc
                    off = 128 * j if j > 0 else 0
                    nc.tensor.matmul(
                        pp[:, half * 512 + off : (half + 1) * 512],
                        kt_sb[:, kt_i * 128 : (kt_i + 1) * 128],
                        qt_sb[:, qc * 512 + off : (qc + 1) * 512],
                        start=True,
                        stop=True,
                    )
                    offs.append((kt_i, j, off))
                p_t = ppool.tile([128, 1024], F16, tag="p_t")
                j0 = offs[0][1]
                if j0 == 2:
                    # (j2,j3) pair: valid columns live in [256:512] of each
                    # half; exp just those strips via a strided view
                    pp_v = pp[:, :].rearrange("p (two q) -> p two q", two=2)
                    pt_v = p_t[:, :].rearrange("p (two q) -> p two q", two=2)
                    nc.scalar.activation(
                        out=pt_v[:, :, 256:],
                        in_=pp_v[:, :, 256:],
                        func=mybir.ActivationFunctionType.Exp,
                        scale=float(SCALE),
                    )
                else:
                    nc.scalar.activation(
                        out=p_t,
                        in_=pp,
                        func=mybir.ActivationFunctionType.Exp,
                        scale=float(SCALE),
                    )
                if j0 >= 0:
                    # zero the causally-invalid region of both halves
                    # (including exp'd garbage columns): element (k, half, u)
                    # is valid iff u - k - 128*(j0+half) >= 0
                    nc.gpsimd.affine_select(
                        out=p_t,
                        in_=p_t,
                        pattern=[[-128, 2], [1, 512]],
                        compare_op=mybir.AluOpType.is_ge,
                        fill=0.0,
                        base=-128 * j0,
                        channel_multiplier=-1,
                    )
                u["p_t"] = p_t
                u["offs"] = offs

            def ctx(u):
                h, qc = u["h"], u["qc"]
                if u["first"]:
                    state[(h, qc)] = {
                        "psum_c": pctx.tile([128, 512], F32, name="psum_c"),
                        "psum_s": psums.tile([128, 512], F32, name="psum_s"),
                        "acc_started": False,
                    }
                st_ = state[(h, qc)]
                p_t, offs = u["p_t"], u["offs"]
                diag = offs[0][1] >= 0
                for half, (kt_i, j, off) in enumerate(offs):
                    seg = p_t[:, half * 512 + off : (half + 1) * 512]
                    nc.tensor.matmul(
                        st_["psum_c"][:, off:],
                        v_full[:, kt_i, h * 128 : (h + 1) * 128],
                        seg,
                        start=(u["first"] and half == 0),
                        stop=(u["last"] and half == 1),
                    )
                    if diag:
                        # diagonal tiles: denominator strip matmul on the PE
                        nc.tensor.matmul(
                            st_["psum_s"][:, off:],
                            ones128,
                            seg,
                            start=(u["first"] and half == 0),
                            stop=(qc == 0 and u["last"] and half == 1),
                        )
                    else:
                        # full tiles: fp16 accumulation on the DVE (exact:
                        # one ones-matmul at finalize sums the partitions)
                        full = p_t[:, half * 512 : (half + 1) * 512]
                        if not st_["acc_started"]:
                            st_["p_acc"] = paccp.tile([128, 512], F16,
                                                      tag="pacc", name="p_acc")
                            nc.vector.tensor_copy(out=st_["p_acc"], in_=full)
                            st_["acc_started"] = True
                        else:
                            nc.vector.tensor_tensor(
                                out=st_["p_acc"],
                                in0=st_["p_acc"],
                                in1=full,
                                op=mybir.AluOpType.add,
                            )

            def finalize(h, qc):
                st_ = state.pop((h, qc))
                if st_["acc_started"]:
                    nc.tensor.matmul(
                        st_["psum_s"],
                        ones128,
                        st_["p_acc"],
                        start=False,
                        stop=True,
                    )
                recip = rcpool.tile([128, 512], F32, tag="rcp")
                nc.vector.reciprocal_approx_fast(out=recip, in_=st_["psum_s"])
                nc.vector.tensor_tensor(
                    out=ct_full[:, h, qc * 512 : (qc + 1) * 512],
                    in0=st_["psum_c"],
                    in1=recip,
                    op=mybir.AluOpType.mult,
                )

            for i, u in enumerate(units):
                if u["first"] and u["qc"] == 1:
                    load_head(u["h"] + 1)  # prefetch next head's q/k early
                if u["first"] and u["qc"] == 0:
                    # spread the 4MB Wo load through phase B
                    nc.sync.dma_start(out=wo_sb[:, u["h"], :],
                                      in_=wo_r[:, u["h"], :])
                scores(u)
                while finalizes and finalizes[0][0] <= i:
                    _, fh, fqc = finalizes.pop(0)
                    finalize(fh, fqc)
                if i > 0:
                    pu = units[i - 1]
                    ctx(pu)
                    if pu["last"]:
                        finalizes.append((i + 2, pu["h"], pu["qc"]))
            ctx(units[-1])
            finalizes.append((0, units[-1]["h"], units[-1]["qc"]))
            for _, fh, fqc in finalizes:
                finalize(fh, fqc)

        # ---------------- Phase C: output projection -------------------------
        with (
            tc.tile_pool(name="opsum", bufs=4, space="PSUM") as ops,
            tc.tile_pool(name="ostage", bufs=4) as ost,
        ):
            for st in range(ST):
                for ncol in range(4):
                    psum = ops.tile([128, 512], F32)
                    for hh in range(HPC):
                        nc.tensor.matmul(
                            psum,
                            ct_full[:, hh, st * 128 : (st + 1) * 128],
                            wo_sb[:, hh, ncol * 512 : (ncol + 1) * 512],
                            start=(hh == 0),
                            stop=(hh == HPC - 1),
                        )
                    o_sb = ost.tile([128, 512], F32, tag="ostage")
                    nc.scalar.activation(
                        out=o_sb, in_=psum, func=mybir.ActivationFunctionType.Copy
                    )
                    nc.gpsimd.dma_start(
                        out=out[
                            st * 128 : (st + 1) * 128,
                            ncol * 512 : (ncol + 1) * 512,
                        ],
                        in_=o_sb,
                    )
        wop_cm.__exit__(None, None, None)
        ctfull_cm.__exit__(None, None, None)


_NC = None


def _get_nc():
    global _NC
    if _NC is None:
        _NC = _build_nc()
    return _NC


def _host_prep(input_sequences, Wq, bq, Wk, bk, Wv, bv, Wo, bo):
    """Build per-core input maps."""
    x = np.asarray(input_sequences, dtype=np.float32)

    in_maps = []
    for c in range(8):
        b, g = divmod(c, 2)
        sl = slice(g * DHG, (g + 1) * DHG)
        wq_c = np.ascontiguousarray(
            np.asarray(Wq[:, sl], dtype=np.float32)
            .reshape(KT, 128, HPC, 128).transpose(2, 1, 0, 3).reshape(DHG, D)
        ).astype(np.float16)
        wk_c = np.ascontiguousarray(
            np.asarray(Wk[:, sl], dtype=np.float32)
            .reshape(KT, 128, HPC, 128).transpose(2, 1, 0, 3).reshape(DHG, D)
        ).astype(np.float16)
        wv_c = np.ascontiguousarray(Wv[:, sl]).astype(np.float16)
        wo_c = np.ascontiguousarray(Wo[sl, :]).astype(np.float16)
        in_maps.append({
            "xT": np.ascontiguousarray(x[b].T).astype(np.float16),
            "wq": wq_c,
            "wk": wk_c,
            "wv": wv_c,
            "wo": wo_c,
            "bqT": np.ascontiguousarray(
                np.asarray(bq[sl], dtype=np.float32).reshape(HPC, 128).T
            ),
            "bkT": np.ascontiguousarray(
                np.asarray(bk[sl], dtype=np.float32).reshape(HPC, 128).T
            ),
            "bvb": np.ascontiguousarray(
                np.broadcast_to(np.asarray(bv[sl], dtype=np.float32), (128, DHG))
            ),
        })
    return in_maps


def kernel(input_sequences, Wq, bq, Wk, bk, Wv, bv, Wo, bo, _trace=False):
    nc = _get_nc()
    in_maps = _host_prep(input_sequences, Wq, bq, Wk, bk, Wv, bv, Wo, bo)
    res = run_bass_kernel_spmd(nc, in_maps, list(range(8)), trace=_trace)
    bo32 = np.asarray(bo, dtype=np.float32)
    out = np.empty((B, S, D), dtype=np.float32)
    for b in range(B):
        out[b] = res.results[2 * b]["out"] + res.results[2 * b + 1]["out"] + bo32
    if _trace:
        kernel.last_exec_time_ns = res.exec_time_ns
    return out


# revision 31
# speedup vs baseline: 1.0027x; 1.0027x over previous
"""Causal multi-head attention on 8 trn2 NeuronCores.

Problem: B=4, S=2048, D=2048, H=16 heads, head_dim=128, causal softmax,
torch-style Linear projections (W stored [in, out]).

Sharding: core c handles batch b = c//2 and head-group g = c%2
(8 heads = 1024 output columns of Wq/Wk/Wv, 1024 rows of Wo).
Each core produces a partial output [S, D]; host sums the two
head-group partials per batch and adds bo.

All matmul operands fp16 (PSUM accumulation stays fp32): fp16 streams
at 1 row/cycle on the PE vs fp32r's ~1.25, and halves DMA/SBUF bytes.

Per-core device pipeline:
  Phase A: from xT (host-pretransposed [D, S]) compute
           Q^T, K^T [1024, S] (spilled to DRAM scratch) and V [S, 1024]
           (kept SBUF-resident: 32KB/partition in fp16).
  Phase B: per head h, per 512-wide q-chunk: k-tile PAIRS:
           scores^T strips [128 k, 512 q] = K_h Q_h^T into a 2-bank
           psum pair, one exp per pair (psum -> fp16 SBUF, no mask),
           causal upper-triangle + garbage zeroed by one gpsimd
           affine_select per diagonal pair (exact: zeros contribute
           nothing downstream), ctx^T accumulation C^T = V_h^T @ P^T,
           denominator accumulated on DVE in fp16 (one ones-matmul per
           q-chunk instead of one per tile), normalize into
           SBUF-resident C^T (32KB/partition).
  Phase C: out_partial = C @ Wo_slice straight from SBUF ct tiles.
"""

import numpy as np

import concourse.bass as bass
import concourse.mybir as mybir
import concourse.tile as tile
from concourse import bacc
from concourse.bass_utils import run_bass_kernel_spmd

B = 4
S = 2048
D = 2048
H = 16
DH = 128
HPC = 8          # heads per core
DHG = HPC * DH   # 1024: head-group width per core
KT = D // 128    # 16 k-tiles over the model dim
ST = S // 128    # 16 s-tiles
QC = S // 512    # 4 q-chunks
SCALE = 1.0 / np.sqrt(DH)

F32 = mybir.dt.float32
F16 = mybir.dt.float16


def _build_nc():
    nc = bacc.Bacc(None, target_bir_lowering=False)

    xT = nc.declare_dram_parameter("xT", [D, S], F16, isOutput=False)
    # wq/wk host-pregathered to [HPC*128, KT*128]: row t*128+p, col n*128+m
    # = Wq[n*128+p, t*128+m] so each head-tile's weights DMA contiguously
    wq = nc.declare_dram_parameter("wq", [DHG, D], F16, isOutput=False)
    wk = nc.declare_dram_parameter("wk", [DHG, D], F16, isOutput=False)
    wv = nc.declare_dram_parameter("wv", [D, DHG], F16, isOutput=False)
    wo = nc.declare_dram_parameter("wo", [DHG, D], F16, isOutput=False)
    bqT = nc.declare_dram_parameter("bqT", [128, HPC], F32, isOutput=False)
    bkT = nc.declare_dram_parameter("bkT", [128, HPC], F32, isOutput=False)
    bvb = nc.declare_dram_parameter("bvb", [128, DHG], F32, isOutput=False)
    out = nc.declare_dram_parameter("out", [S, D], F32, isOutput=True)

    with tile.TileContext(nc) as tc:
        _emit(nc, tc, xT, wq, wk, wv, wo, bqT, bkT, bvb, out)
    nc.compile()
    return nc


def _emit(nc, tc, xT, wq, wk, wv, wo, bqT, bkT, bvb, out):
    with (
        tc.tile_pool(name="const", bufs=1) as const,
        tc.tile_pool(name="dram", bufs=1, space="DRAM") as dram,
        tc.tile_pool(name="qkpre", bufs=4) as qkpre,
        tc.tile_pool(name="vfull", bufs=1) as vfull,
    ):
        qt_d = dram.tile([DHG, S], F16)
        kt_d = dram.tile([DHG, S], F16)

        bq_sb = const.tile([128, HPC], F32)
        bk_sb = const.tile([128, HPC], F32)
        bv_sb = const.tile([128, DHG], F32)
        ones_f32 = const.tile([128, 128], F32)
        nc.vector.memset(ones_f32, 1.0)
        ones128 = const.tile([128, 128], F16)
        nc.vector.tensor_copy(out=ones128, in_=ones_f32)

        # V [S, DHG] stays in SBUF for the whole kernel (ct_full and wo_sb
        # allocate lazily at phase B so their space is free during phase A)
        v_full = vfull.tile([128, ST, DHG], F16)

        head0 = {}  # head-0 q/k tiles, prefetched during phase A

        # ---------------- Phase A: projections -------------------------------
        wv_r = wv.rearrange("(n p) m -> p n m", p=128)

        with (
            tc.tile_pool(name="xts", bufs=6) as xtp,
            tc.tile_pool(name="wqk", bufs=2) as wqk,
            tc.tile_pool(name="wvp", bufs=1) as wvp,
            tc.tile_pool(name="apsum", bufs=8, space="PSUM") as aps,
            tc.tile_pool(name="astage", bufs=4) as ast,
        ):
            wv_full = wvp.tile([128, KT, DHG], F16)

            for sh in range(2):
                s0 = sh * (S // 2)
                # weight tiles prefetched (depth 2) ahead of the bulk xT DMAs
                seq = [(w, b, dst, t)
                       for w, b, dst in ((wq, bq_sb, qt_d), (wk, bk_sb, kt_d))
                       for t in range(HPC)]
                w_tiles = {}

                def w_prefetch(i):
                    if i < len(seq):
                        w, _, _, t = seq[i]
                        w_sb = wqk.tile([128, KT, 128], F16, tag="wqk",
                                        name=f"w_sb{i % 2}")
                        nc.sync.dma_start(
                            out=w_sb,
                            in_=w[t * 128 : (t + 1) * 128, :]
                            .rearrange("p (n m) -> p n m", m=128),
                        )
                        w_tiles[i] = w_sb

                w_prefetch(0)
                if sh == 0:
                    # bias loads queue behind the critical first weight tile
                    nc.sync.dma_start(out=bq_sb, in_=bqT[:, :])
                    nc.sync.dma_start(out=bk_sb, in_=bkT[:, :])

                # xT half as four sub-tiles of 4 k-tiles each, DMA'd per
                # k-tile: tile-granular dependency tracking then lets the
                # first matmuls start after only a few k-tiles have landed.
                # Loads go on the gpsimd queue so the w_sb prefetch chain on
                # the sync queue is never stuck behind them.
                xt_sub = [
                    xtp.tile([128, 4, S // 2], F16, tag="xts",
                             name=f"xt{sh}{q}")
                    for q in range(4)
                ]

                def xt_blk(kd):
                    return xt_sub[kd // 4][:, kd % 4, :]

                # alternate issue queues: 16 issues at ~640ns each on one
                # queue would feed tiles slower than the first QK sweep
                # consumes them
                for kd in range(KT):
                    eng = nc.gpsimd if kd % 2 == 0 else nc.scalar
                    eng.dma_start(
                        out=xt_blk(kd),
                        in_=xT[kd * 128 : (kd + 1) * 128, s0 : s0 + S // 2],
                    )

                if sh == 0:
                    # bvb is only needed at V time
                    nc.scalar.dma_start(out=bv_sb, in_=bvb[:, :])

                # Q^T and K^T: psum[dh 128, s 512] = sum_kd Wblk^T @ xTblk
                for i, (w, b_sb, dst, t) in enumerate(seq):
                    w_sb = w_tiles.pop(i)
                    w_prefetch(i + 1)
                    for sc in range(2):
                        psum = aps.tile([128, 512], F32, tag="apsum", name="qk_ps")
                        for kd in range(KT):
                            nc.tensor.matmul(
                                psum,
                                w_sb[:, kd, :],
                                xt_blk(kd)[:, sc * 512 : (sc + 1) * 512],
                                start=(kd == 0),
                                stop=(kd == KT - 1),
                            )
                        stg = ast.tile([128, 512], F16, tag="astage")
                        nc.vector.tensor_scalar_add(
                            out=stg, in0=psum, scalar1=b_sb[:, t : t + 1]
                        )
                        nc.gpsimd.dma_start(
                            out=dst[
                                t * 128 : (t + 1) * 128,
                                s0 + sc * 512 : s0 + (sc + 1) * 512,
                            ],
                            in_=stg,
                        )
                    if sh == 0 and i == 3:
                        # 4MB wv load on the gpsimd queue, emitted behind the
                        # first QK spills: the spill dma_starts wait on their
                        # compute, so wv's ~15us descriptor flood hits the
                        # rings only after the latency-critical xT tiles have
                        # all landed. V first needs wv ~70us later.
                        nc.gpsimd.dma_start(out=wv_full, in_=wv_r[:, :, :])

                if sh == 1:
                    # heads 0 and 1 both start immediately in the two-head
                    # interleaved phase B: prefetch both during phase A
                    for hh in range(2):
                        qt0 = qkpre.tile([128, S], F16, tag="qt",
                                         name=f"qt0{hh}")
                        nc.sync.dma_start(
                            out=qt0, in_=qt_d[hh * 128 : (hh + 1) * 128, :])
                        kt0 = qkpre.tile([128, S], F16, tag="kt",
                                         name=f"kt0{hh}")
                        nc.sync.dma_start(
                            out=kt0, in_=kt_d[hh * 128 : (hh + 1) * 128, :])
                        head0[hh] = (qt0, kt0)

                # V: psum[s 128, dh 512] = sum_kd xTblk^T @ Wvblk; bias-added
                # result written straight into the SBUF-resident v_full.
                # si-groups of 4 so each group's DVE drain overlaps the next
                # group's matmul sweep (and the tail drain is short).
                for t2 in range(2):
                    for sg in range(2):
                        psums = [
                            aps.tile([128, 512], F32, tag="apsum",
                                     name=f"vps{si}")
                            for si in range(4)
                        ]
                        for kd in range(KT):
                            for si in range(4):
                                nc.tensor.matmul(
                                    psums[si],
                                    xt_blk(kd)[:, (sg * 4 + si) * 128 :
                                               (sg * 4 + si + 1) * 128],
                                    wv_full[:, kd, t2 * 512 : (t2 + 1) * 512],
                                    start=(kd == 0),
                                    stop=(kd == KT - 1),
                                )
                        for si in range(4):
                            nc.vector.tensor_tensor(
                                out=v_full[:, sh * 8 + sg * 4 + si,
                                           t2 * 512 : (t2 + 1) * 512],
                                in0=psums[si],
                                in1=bv_sb[:, t2 * 512 : (t2 + 1) * 512],
                                op=mybir.AluOpType.add,
                            )

        # ---------------- Phase B: per-head attention ------------------------
        wo_r = wo.rearrange("(n p) m -> p n m", p=128)
        # ct_full / wo_sb pools open only now (span B and C) so their 64KB
        # per partition is free during phase A
        ctfull_cm = tc.tile_pool(name="ctfull", bufs=1)
        ctfull = ctfull_cm.__enter__()
        wop_cm = tc.tile_pool(name="wop", bufs=1)
        wop = wop_cm.__enter__()
        with (
            tc.tile_pool(name="ptile", bufs=5) as ppool,
            tc.tile_pool(name="pacc", bufs=2) as paccp,
            tc.tile_pool(name="rcp", bufs=2) as rcpool,
            tc.tile_pool(name="pscore", bufs=2, space="PSUM") as pscore,
            tc.tile_pool(name="pctx", bufs=2, space="PSUM") as pctx,
            tc.tile_pool(name="psum2", bufs=2, space="PSUM") as psums,
        ):
            ct_full = ctfull.tile([128, HPC, S], F16)
            wo_sb = wop.tile([128, HPC, D], F16)

            # Flat pipeline over pair-units, with TWO heads interleaved:
            # consecutive units belong to different heads, so every
            # scores->exp->select->ctx/acc chain has a full unit of
            # independent work in front of it and the PE never drains at a
            # qc boundary.  Per-qc finalization (denominator matmul + recip
            # + normalize) is deferred two units so its inputs are ready.
            def head_units(h):
                us = []
                for qc in range(QC):
                    prs = [(4 * qc, 4 * qc + 1), (4 * qc + 2, 4 * qc + 3)]
                    prs += [(t, t + 1) for t in range(0, 4 * qc, 2)]
                    for pi, pair in enumerate(prs):
                        us.append({
                            "h": h, "qc": qc, "pair": pair,
                            "first": pi == 0, "last": pi == len(prs) - 1,
                        })
                return us

            units = []
            for hp in range(HPC // 2):
                ua, ub = head_units(2 * hp), head_units(2 * hp + 1)
                for a, b in zip(ua, ub):
                    units.append(a)
                    units.append(b)

            qk_tiles = {0: head0[0], 1: head0[1]}

            def load_head(h):
                if h in qk_tiles or h >= HPC:
                    return
                qt_sb = qkpre.tile([128, S], F16, tag="qt", name="qt_sb")
                nc.sync.dma_start(out=qt_sb, in_=qt_d[h * 128 : (h + 1) * 128, :])
                kt_sb = qkpre.tile([128, S], F16, tag="kt", name="kt_sb")
                nc.sync.dma_start(out=kt_sb, in_=kt_d[h * 128 : (h + 1) * 128, :])
                qk_tiles[h] = (qt_sb, kt_sb)

            state = {}      # (h, qc) -> dict with psum_c, psum_s, p_acc
            finalizes = []  # (due_iteration, h, qc)

            def scores(u):
                h, qc = u["h"], u["qc"]
                qt_sb, kt_sb = qk_tiles[h]
                pp = pscore.tile([128, 1024], F32, tag="pp")
                offs = []
                for half, kt_i in enumerate(u["pair"]):
                    j = kt_i - 4 * qc
                    off = 128 * j if j > 0 else 0
                    nc.tensor.matmul(
                        pp[:, half * 512 + off : (half + 1) * 512],
                        kt_sb[:, kt_i * 128 : (kt_i + 1) * 128],
                        qt_sb[:, qc * 512 + off : (qc + 1) * 512],
                        start=True,
                        stop=True,
                    )
                    offs.append((kt_i, j, off))
                p_t = ppool.tile([128, 1024], F16, tag="p_t")
                j0 = offs[0][1]
                if j0 == 2:
                    # (j2,j3) pair: valid columns live in [256:512] of each
                    # half; exp just those strips via a strided view
                    pp_v = pp[:, :].rearrange("p (two q) -> p two q", two=2)
                    pt_v = p_t[:, :].rearrange("p (two q) -> p two q", two=2)
                    nc.scalar.activation(
                        out=pt_v[:, :, 256:],
                        in_=pp_v[:, :, 256:],
                        func=mybir.ActivationFunctionType.Exp,
                        scale=float(SCALE),
                    )
                else:
                    nc.scalar.activation(
                        out=p_t,
                        in_=pp,
                        func=mybir.ActivationFunctionType.Exp,
                        scale=float(SCALE),
                    )
                if j0 >= 0:
                    # zero the causally-invalid region of both halves
                    # (including exp'd garbage columns): element (k, half, u)
                    # is valid iff u - k - 128*(j0+half) >= 0
                    nc.gpsimd.affine_select(
                        out=p_t,
                        in_=p_t,
                        pattern=[[-128, 2], [1, 512]],
                        compare_op=mybir.AluOpType.is_ge,
                        fill=0.0,
                        base=-128 * j0,
                        channel_multiplier=-1,
                    )
                u["p_t"] = p_t
                u["offs"] = offs

            def ctx(u):
                h, qc = u["h"], u["qc"]
                if u["first"]:
                    state[(h, qc)] = {
                        "psum_c": pctx.tile([128, 512], F32, name="psum_c"),
                        "psum_s": psums.tile([128, 512], F32, name="psum_s"),
                        "acc_started": False,
                    }
                st_ = state[(h, qc)]
                p_t, offs = u["p_t"], u["offs"]
                diag = offs[0][1] >= 0
                for half, (kt_i, j, off) in enumerate(offs):
                    seg = p_t[:, half * 512 + off : (half + 1) * 512]
                    nc.tensor.matmul(
                        st_["psum_c"][:, off:],
                        v_full[:, kt_i, h * 128 : (h + 1) * 128],
                        seg,
                        start=(u["first"] and half == 0),
                        stop=(u["last"] and half == 1),
                    )
                    if diag:
                        # diagonal tiles: denominator strip matmul on the PE
                        nc.tensor.matmul(
                            st_["psum_s"][:, off:],
                            ones128,
                            seg,
                            start=(u["first"] and half == 0),
                            stop=(qc == 0 and u["last"] and half == 1),
                        )
                    else:
                        # full tiles: fp16 accumulation on the DVE (exact:
                        # one ones-matmul at finalize sums the partitions)
                        full = p_t[:, half * 512 : (half + 1) * 512]
                        if not st_["acc_started"]:
                            st_["p_acc"] = paccp.tile([128, 512], F16,
                                                      tag="pacc", name="p_acc")
                            nc.vector.tensor_copy(out=st_["p_acc"], in_=full)
                            st_["acc_started"] = True
                        else:
                            nc.vector.tensor_tensor(
                                out=st_["p_acc"],
                                in0=st_["p_acc"],
                                in1=full,
                                op=mybir.AluOpType.add,
                            )

            def finalize(h, qc):
                st_ = state.pop((h, qc))
                if st_["acc_started"]:
                    nc.tensor.matmul(
                        st_["psum_s"],
                        ones128,
                        st_["p_acc"],
                        start=False,
                        stop=True,
                    )
                recip = rcpool.tile([128, 512], F32, tag="rcp")
                nc.vector.reciprocal_approx_fast(out=recip, in_=st_["psum_s"])
                nc.vector.tensor_tensor(
                    out=ct_full[:, h, qc * 512 : (qc + 1) * 512],
                    in0=st_["psum_c"],
                    in1=recip,
                    op=mybir.AluOpType.mult,
                )

            for i, u in enumerate(units):
                if u["first"] and u["h"] % 2 == 0 and u["qc"] in (1, 2):
                    # prefetch the next head pair's q/k early
                    load_head(u["h"] + 1 + u["qc"])
                if u["first"] and u["qc"] == 0:
                    # spread the 4MB Wo load through phase B
                    nc.sync.dma_start(out=wo_sb[:, u["h"], :],
                                      in_=wo_r[:, u["h"], :])
                scores(u)
                while finalizes and finalizes[0][0] <= i:
                    _, fh, fqc = finalizes.pop(0)
                    finalize(fh, fqc)
                if i > 0:
                    pu = units[i - 1]
                    ctx(pu)
                    if pu["last"]:
                        finalizes.append((i + 2, pu["h"], pu["qc"]))
            ctx(units[-1])
            finalizes.append((0, units[-1]["h"], units[-1]["qc"]))
            for _, fh, fqc in finalizes:
                finalize(fh, fqc)

        # ---------------- Phase C: output projection -------------------------
        with (
            tc.tile_pool(name="opsum", bufs=4, space="PSUM") as ops,
            tc.tile_pool(name="ostage", bufs=4) as ost,
        ):
            for st in range(ST):
                for ncol in range(4):
                    psum = ops.tile([128, 512], F32)
                    for hh in range(HPC):
                        nc.tensor.matmul(
                            psum,
                            ct_full[:, hh, st * 128 : (st + 1) * 128],
                            wo_sb[:, hh, ncol * 512 : (ncol + 1) * 512],
                            start=(hh == 0),
                            stop=(hh == HPC - 1),
                        )
                    o_sb = ost.tile([128, 512], F32, tag="ostage")
                    nc.scalar.activation(
                        out=o_sb, in_=psum, func=mybir.ActivationFunctionType.Copy
                    )
                    nc.gpsimd.dma_start(
                        out=out[
                            st * 128 : (st + 1) * 128,
                            ncol * 512 : (ncol + 1) * 512,
                        ],
                        in_=o_sb,
                    )
        wop_cm.__exit__(None, None, None)
        ctfull_cm.__exit__(None, None, None)


_NC = None


def _get_nc():
    global _NC
    if _NC is None:
        _NC = _build_nc()
    return _NC


def _host_prep(input_sequences, Wq, bq, Wk, bk, Wv, bv, Wo, bo):
    """Build per-core input maps."""
    x = np.asarray(input_sequences, dtype=np.float32)

    in_maps = []
    for c in range(8):
        b, g = divmod(c, 2)
        sl = slice(g * DHG, (g + 1) * DHG)
        wq_c = np.ascontiguousarray(
            np.asarray(Wq[:, sl], dtype=np.float32)
            .reshape(KT, 128, HPC, 128).transpose(2, 1, 0, 3).reshape(DHG, D)
        ).astype(np.float16)
        wk_c = np.ascontiguousarray(
            np.asarray(Wk[:, sl], dtype=np.float32)
            .reshape(KT, 128, HPC, 128).transpose(2, 1, 0, 3).reshape(DHG, D)
        ).astype(np.float16)
        wv_c = np.ascontiguousarray(Wv[:, sl]).astype(np.float16)
        wo_c = np.ascontiguousarray(Wo[sl, :]).astype(np.float16)
        in_maps.append({
            "xT": np.ascontiguousarray(x[b].T).astype(np.float16),
            "wq": wq_c,
            "wk": wk_c,
            "wv": wv_c,
            "wo": wo_c,
            "bqT": np.ascontiguousarray(
                np.asarray(bq[sl], dtype=np.float32).reshape(HPC, 128).T
            ),
            "bkT": np.ascontiguousarray(
                np.asarray(bk[sl], dtype=np.float32).reshape(HPC, 128).T
            ),
            "bvb": np.ascontiguousarray(
                np.broadcast_to(np.asarray(bv[sl], dtype=np.float32), (128, DHG))
            ),
        })
    return in_maps


def kernel(input_sequences, Wq, bq, Wk, bk, Wv, bv, Wo, bo, _trace=False):
    nc = _get_nc()
    in_maps = _host_prep(input_sequences, Wq, bq, Wk, bk, Wv, bv, Wo, bo)
    res = run_bass_kernel_spmd(nc, in_maps, list(range(8)), trace=_trace)
    bo32 = np.asarray(bo, dtype=np.float32)
    out = np.empty((B, S, D), dtype=np.float32)
    for b in range(B):
        out[b] = res.results[2 * b]["out"] + res.results[2 * b + 1]["out"] + bo32
    if _trace:
        kernel.last_exec_time_ns = res.exec_time_ns
    return out


# revision 34
# speedup vs baseline: 1.0097x; 1.0069x over previous
"""Causal multi-head attention on 8 trn2 NeuronCores.

Problem: B=4, S=2048, D=2048, H=16 heads, head_dim=128, causal softmax,
torch-style Linear projections (W stored [in, out]).

Sharding: core c handles batch b = c//2 and head-group g = c%2
(8 heads = 1024 output columns of Wq/Wk/Wv, 1024 rows of Wo).
Each core produces a partial output [S, D]; host sums the two
head-group partials per batch and adds bo.

All matmul operands fp16 (PSUM accumulation stays fp32): fp16 streams
at 1 row/cycle on the PE vs fp32r's ~1.25, and halves DMA/SBUF bytes.

Per-core device pipeline:
  Phase A: from xT (host-pretransposed [D, S]) compute
           Q^T, K^T [1024, S] (spilled to DRAM scratch) and V [S, 1024]
           (kept SBUF-resident: 32KB/partition in fp16).
  Phase B: per head h, per 512-wide q-chunk: k-tile PAIRS:
           scores^T strips [128 k, 512 q] = K_h Q_h^T into a 2-bank
           psum pair, one exp per pair (psum -> fp16 SBUF, no mask),
           causal upper-triangle + garbage zeroed by one gpsimd
           affine_select per diagonal pair (exact: zeros contribute
           nothing downstream), ctx^T accumulation C^T = V_h^T @ P^T,
           denominator accumulated on DVE in fp16 (one ones-matmul per
           q-chunk instead of one per tile), normalize into
           SBUF-resident C^T (32KB/partition).
  Phase C: out_partial = C @ Wo_slice straight from SBUF ct tiles.
"""

import numpy as np

import concourse.bass as bass
import concourse.mybir as mybir
import concourse.tile as tile
from concourse import bacc
from concourse.bass_utils import run_bass_kernel_spmd

B = 4
S = 2048
D = 2048
H = 16
DH = 128
HPC = 8          # heads per core
DHG = HPC * DH   # 1024: head-group width per core
KT = D // 128    # 16 k-tiles over the model dim
ST = S // 128    # 16 s-tiles
QC = S // 512    # 4 q-chunks
SCALE = 1.0 / np.sqrt(DH)

F32 = mybir.dt.float32
F16 = mybir.dt.float16


def _build_nc():
    nc = bacc.Bacc(None, target_bir_lowering=False)

    xT = nc.declare_dram_parameter("xT", [D, S], F16, isOutput=False)
    # wq/wk host-pregathered to [HPC*128, KT*128]: row t*128+p, col n*128+m
    # = Wq[n*128+p, t*128+m] so each head-tile's weights DMA contiguously
    wq = nc.declare_dram_parameter("wq", [DHG, D], F16, isOutput=False)
    wk = nc.declare_dram_parameter("wk", [DHG, D], F16, isOutput=False)
    wv = nc.declare_dram_parameter("wv", [D, DHG], F16, isOutput=False)
    wo = nc.declare_dram_parameter("wo", [DHG, D], F16, isOutput=False)
    bqT = nc.declare_dram_parameter("bqT", [128, HPC], F32, isOutput=False)
    bkT = nc.declare_dram_parameter("bkT", [128, HPC], F32, isOutput=False)
    bvb = nc.declare_dram_parameter("bvb", [128, DHG], F32, isOutput=False)
    out = nc.declare_dram_parameter("out", [S, D], F32, isOutput=True)

    with tile.TileContext(nc) as tc:
        _emit(nc, tc, xT, wq, wk, wv, wo, bqT, bkT, bvb, out)
    nc.compile()
    return nc


def _emit(nc, tc, xT, wq, wk, wv, wo, bqT, bkT, bvb, out):
    with (
        tc.tile_pool(name="const", bufs=1) as const,
        tc.tile_pool(name="dram", bufs=1, space="DRAM") as dram,
        tc.tile_pool(name="qkpre", bufs=2) as qkpre,
        tc.tile_pool(name="vfull", bufs=1) as vfull,
    ):
        qt_d = dram.tile([DHG, S], F16)
        kt_d = dram.tile([DHG, S], F16)

        bq_sb = const.tile([128, HPC], F32)
        bk_sb = const.tile([128, HPC], F32)
        bv_sb = const.tile([128, DHG], F32)
        ones_f32 = const.tile([128, 128], F32)
        nc.vector.memset(ones_f32, 1.0)
        ones128 = const.tile([128, 128], F16)
        nc.vector.tensor_copy(out=ones128, in_=ones_f32)

        # V [S, DHG] stays in SBUF for the whole kernel (ct_full and wo_sb
        # allocate lazily at phase B so their space is free during phase A)
        v_full = vfull.tile([128, ST, DHG], F16)

        head0 = {}  # head-0 q/k tiles, prefetched during phase A

        # ---------------- Phase A: projections -------------------------------
        wv_r = wv.rearrange("(n p) m -> p n m", p=128)

        with (
            tc.tile_pool(name="xts", bufs=6) as xtp,
            tc.tile_pool(name="wqk", bufs=2) as wqk,
            tc.tile_pool(name="wvp", bufs=1) as wvp,
            tc.tile_pool(name="apsum", bufs=8, space="PSUM") as aps,
            tc.tile_pool(name="astage", bufs=4) as ast,
        ):
            wv_full = wvp.tile([128, KT, DHG], F16)

            for sh in range(2):
                s0 = sh * (S // 2)
                # weight tiles prefetched (depth 2) ahead of the bulk xT DMAs
                seq = [(w, b, dst, t)
                       for w, b, dst in ((wq, bq_sb, qt_d), (wk, bk_sb, kt_d))
                       for t in range(HPC)]
                w_tiles = {}

                def w_prefetch(i):
                    if i < len(seq):
                        w, _, _, t = seq[i]
                        w_sb = wqk.tile([128, KT, 128], F16, tag="wqk",
                                        name=f"w_sb{i % 2}")
                        nc.sync.dma_start(
                            out=w_sb,
                            in_=w[t * 128 : (t + 1) * 128, :]
                            .rearrange("p (n m) -> p n m", m=128),
                        )
                        w_tiles[i] = w_sb

                w_prefetch(0)
                if sh == 0:
                    # bias loads queue behind the critical first weight tile
                    nc.sync.dma_start(out=bq_sb, in_=bqT[:, :])
                    nc.sync.dma_start(out=bk_sb, in_=bkT[:, :])

                # xT half as four sub-tiles of 4 k-tiles each, DMA'd per
                # k-tile: tile-granular dependency tracking then lets the
                # first matmuls start after only a few k-tiles have landed.
                # Loads go on the gpsimd queue so the w_sb prefetch chain on
                # the sync queue is never stuck behind them.
                xt_sub = [
                    xtp.tile([128, 4, S // 2], F16, tag="xts",
                             name=f"xt{sh}{q}")
                    for q in range(4)
                ]

                def xt_blk(kd):
                    return xt_sub[kd // 4][:, kd % 4, :]

                # alternate issue queues: 16 issues at ~640ns each on one
                # queue would feed tiles slower than the first QK sweep
                # consumes them
                for kd in range(KT):
                    eng = nc.gpsimd if kd % 2 == 0 else nc.scalar
                    eng.dma_start(
                        out=xt_blk(kd),
                        in_=xT[kd * 128 : (kd + 1) * 128, s0 : s0 + S // 2],
                    )

                if sh == 0:
                    # bvb is only needed at V time
                    nc.scalar.dma_start(out=bv_sb, in_=bvb[:, :])

                # Q^T and K^T: psum[dh 128, s 512] = sum_kd Wblk^T @ xTblk
                for i, (w, b_sb, dst, t) in enumerate(seq):
                    w_sb = w_tiles.pop(i)
                    w_prefetch(i + 1)
                    for sc in range(2):
                        psum = aps.tile([128, 512], F32, tag="apsum", name="qk_ps")
                        for kd in range(KT):
                            nc.tensor.matmul(
                                psum,
                                w_sb[:, kd, :],
                                xt_blk(kd)[:, sc * 512 : (sc + 1) * 512],
                                start=(kd == 0),
                                stop=(kd == KT - 1),
                            )
                        stg = ast.tile([128, 512], F16, tag="astage")
                        nc.vector.tensor_scalar_add(
                            out=stg, in0=psum, scalar1=b_sb[:, t : t + 1]
                        )
                        nc.gpsimd.dma_start(
                            out=dst[
                                t * 128 : (t + 1) * 128,
                                s0 + sc * 512 : s0 + (sc + 1) * 512,
                            ],
                            in_=stg,
                        )
                    if sh == 0 and i == 3:
                        # 4MB wv load on the gpsimd queue, emitted behind the
                        # first QK spills: the spill dma_starts wait on their
                        # compute, so wv's ~15us descriptor flood hits the
                        # rings only after the latency-critical xT tiles have
                        # all landed. V first needs wv ~70us later.
                        nc.gpsimd.dma_start(out=wv_full, in_=wv_r[:, :, :])

                if sh == 1:
                    # heads 0 and 1 both start immediately in the two-head
                    # interleaved phase B: prefetch both during phase A
                    for hh in range(2):
                        qt0 = qkpre.tile([128, S], F16, tag="qt",
                                         name=f"qt0{hh}")
                        nc.sync.dma_start(
                            out=qt0, in_=qt_d[hh * 128 : (hh + 1) * 128, :])
                        kt0 = qkpre.tile([128, S], F16, tag="kt",
                                         name=f"kt0{hh}")
                        nc.sync.dma_start(
                            out=kt0, in_=kt_d[hh * 128 : (hh + 1) * 128, :])
                        head0[hh] = (qt0, kt0)

                # V: psum[s 128, dh 512] = sum_kd xTblk^T @ Wvblk; bias-added
                # result written straight into the SBUF-resident v_full.
                # si-groups of 4 so each group's DVE drain overlaps the next
                # group's matmul sweep (and the tail drain is short).
                for t2 in range(2):
                    for sg in range(2):
                        psums = [
                            aps.tile([128, 512], F32, tag="apsum",
                                     name=f"vps{si}")
                            for si in range(4)
                        ]
                        for kd in range(KT):
                            for si in range(4):
                                nc.tensor.matmul(
                                    psums[si],
                                    xt_blk(kd)[:, (sg * 4 + si) * 128 :
                                               (sg * 4 + si + 1) * 128],
                                    wv_full[:, kd, t2 * 512 : (t2 + 1) * 512],
                                    start=(kd == 0),
                                    stop=(kd == KT - 1),
                                )
                        for si in range(4):
                            nc.vector.tensor_tensor(
                                out=v_full[:, sh * 8 + sg * 4 + si,
                                           t2 * 512 : (t2 + 1) * 512],
                                in0=psums[si],
                                in1=bv_sb[:, t2 * 512 : (t2 + 1) * 512],
                                op=mybir.AluOpType.add,
                            )

        # ---------------- Phase B: per-head attention ------------------------
        wo_r = wo.rearrange("(n p) m -> p n m", p=128)
        # ct_full / wo_sb pools open only now (span B and C) so their 64KB
        # per partition is free during phase A
        ctfull_cm = tc.tile_pool(name="ctfull", bufs=1)
        ctfull = ctfull_cm.__enter__()
        wop_cm = tc.tile_pool(name="wop", bufs=1)
        wop = wop_cm.__enter__()
        with (
            tc.tile_pool(name="ptile", bufs=4) as ppool,
            tc.tile_pool(name="pacc", bufs=2) as paccp,
            tc.tile_pool(name="rcp", bufs=2) as rcpool,
            tc.tile_pool(name="pscore", bufs=2, space="PSUM") as pscore,
            tc.tile_pool(name="pctx", bufs=2, space="PSUM") as pctx,
            tc.tile_pool(name="psum2", bufs=2, space="PSUM") as psums,
        ):
            ct_full = ctfull.tile([128, HPC, S], F16)
            wo_sb = wop.tile([128, HPC, D], F16)

            # Flat (head, qc, pair) pipeline.  Scores run one pair ahead of
            # ctx, crossing qc/head boundaries so the PE never drains at a
            # boundary; per-qc finalization (denominator matmul + recip +
            # normalize) is deferred two pair-units so its inputs are ready.
            units = []
            for h in range(HPC):
                for qc in range(QC):
                    prs = [(4 * qc, 4 * qc + 1), (4 * qc + 2, 4 * qc + 3)]
                    prs += [(t, t + 1) for t in range(0, 4 * qc, 2)]
                    for pi, pair in enumerate(prs):
                        units.append({
                            "h": h, "qc": qc, "pair": pair,
                            "first": pi == 0, "last": pi == len(prs) - 1,
                        })

            qk_tiles = {0: head0[0], 1: head0[1]}

            def load_head(h):
                if h in qk_tiles or h >= HPC:
                    return
                qt_sb = qkpre.tile([128, S], F16, tag="qt", name="qt_sb")
                nc.sync.dma_start(out=qt_sb, in_=qt_d[h * 128 : (h + 1) * 128, :])
                kt_sb = qkpre.tile([128, S], F16, tag="kt", name="kt_sb")
                nc.sync.dma_start(out=kt_sb, in_=kt_d[h * 128 : (h + 1) * 128, :])
                qk_tiles[h] = (qt_sb, kt_sb)

            state = {}      # (h, qc) -> dict with psum_c, psum_s, p_acc
            finalizes = []  # (due_iteration, h, qc)

            def scores(u):
                h, qc = u["h"], u["qc"]
                qt_sb, kt_sb = qk_tiles[h]
                pp = pscore.tile([128, 1024], F32, tag="pp")
                offs = []
                for half, kt_i in enumerate(u["pair"]):
                    j = kt_i - 4 * qc
                    off = 128 * j if j > 0 else 0
                    nc.tensor.matmul(
                        pp[:, half * 512 + off : (half + 1) * 512],
                        kt_sb[:, kt_i * 128 : (kt_i + 1) * 128],
                        qt_sb[:, qc * 512 + off : (qc + 1) * 512],
                        start=True,
                        stop=True,
                    )
                    offs.append((kt_i, j, off))
                p_t = ppool.tile([128, 1024], F16, tag="p_t")
                j0 = offs[0][1]
                if j0 == 2:
                    # (j2,j3) pair: valid columns live in [256:512] of each
                    # half; exp just those strips via a strided view
                    pp_v = pp[:, :].rearrange("p (two q) -> p two q", two=2)
                    pt_v = p_t[:, :].rearrange("p (two q) -> p two q", two=2)
                    nc.scalar.activation(
                        out=pt_v[:, :, 256:],
                        in_=pp_v[:, :, 256:],
                        func=mybir.ActivationFunctionType.Exp,
                        scale=float(SCALE),
                    )
                else:
                    nc.scalar.activation(
                        out=p_t,
                        in_=pp,
                        func=mybir.ActivationFunctionType.Exp,
                        scale=float(SCALE),
                    )
                if j0 >= 0:
                    # zero the causally-invalid region of both halves
                    # (including exp'd garbage columns): element (k, half, u)
                    # is valid iff u - k - 128*(j0+half) >= 0
                    nc.gpsimd.affine_select(
                        out=p_t,
                        in_=p_t,
                        pattern=[[-128, 2], [1, 512]],
                        compare_op=mybir.AluOpType.is_ge,
                        fill=0.0,
                        base=-128 * j0,
                        channel_multiplier=-1,
                    )
                u["p_t"] = p_t
                u["offs"] = offs

            def ctx(u):
                h, qc = u["h"], u["qc"]
                if u["first"]:
                    state[(h, qc)] = {
                        "psum_c": pctx.tile([128, 512], F32, name="psum_c"),
                        "psum_s": psums.tile([128, 512], F32, name="psum_s"),
                        "acc_started": False,
                    }
                st_ = state[(h, qc)]
                p_t, offs = u["p_t"], u["offs"]
                diag = offs[0][1] >= 0
                for half, (kt_i, j, off) in enumerate(offs):
                    seg = p_t[:, half * 512 + off : (half + 1) * 512]
                    nc.tensor.matmul(
                        st_["psum_c"][:, off:],
                        v_full[:, kt_i, h * 128 : (h + 1) * 128],
                        seg,
                        start=(u["first"] and half == 0),
                        stop=(u["last"] and half == 1),
                    )
                    if diag:
                        # diagonal tiles: denominator strip matmul on the PE
                        nc.tensor.matmul(
                            st_["psum_s"][:, off:],
                            ones128,
                            seg,
                            start=(u["first"] and half == 0),
                            stop=(qc == 0 and u["last"] and half == 1),
                        )
                    else:
                        # full tiles: fp16 accumulation on the DVE (exact:
                        # one ones-matmul at finalize sums the partitions)
                        full = p_t[:, half * 512 : (half + 1) * 512]
                        if not st_["acc_started"]:
                            st_["p_acc"] = paccp.tile([128, 512], F16,
                                                      tag="pacc", name="p_acc")
                            nc.vector.tensor_copy(out=st_["p_acc"], in_=full)
                            st_["acc_started"] = True
                        else:
                            nc.vector.tensor_tensor(
                                out=st_["p_acc"],
                                in0=st_["p_acc"],
                                in1=full,
                                op=mybir.AluOpType.add,
                            )

            def finalize(h, qc):
                st_ = state.pop((h, qc))
                if st_["acc_started"]:
                    nc.tensor.matmul(
                        st_["psum_s"],
                        ones128,
                        st_["p_acc"],
                        start=False,
                        stop=True,
                    )
                recip = rcpool.tile([128, 512], F32, tag="rcp")
                nc.vector.reciprocal_approx_fast(out=recip, in_=st_["psum_s"])
                nc.vector.tensor_tensor(
                    out=ct_full[:, h, qc * 512 : (qc + 1) * 512],
                    in0=st_["psum_c"],
                    in1=recip,
                    op=mybir.AluOpType.mult,
                )

            for i, u in enumerate(units):
                if u["first"] and u["qc"] == 1:
                    load_head(u["h"] + 1)  # prefetch next head's q/k early
                if u["first"] and u["qc"] == 0:
                    # spread the 4MB Wo load through phase B
                    nc.sync.dma_start(out=wo_sb[:, u["h"], :],
                                      in_=wo_r[:, u["h"], :])
                scores(u)
                while finalizes and finalizes[0][0] <= i:
                    _, fh, fqc = finalizes.pop(0)
                    finalize(fh, fqc)
                if i > 0:
                    pu = units[i - 1]
                    ctx(pu)
                    if pu["last"]:
                        finalizes.append((i + 2, pu["h"], pu["qc"]))
            ctx(units[-1])
            finalizes.append((0, units[-1]["h"], units[-1]["qc"]))
            for _, fh, fqc in finalizes:
                finalize(fh, fqc)

        # ---------------- Phase C: output projection -------------------------
        with (
            tc.tile_pool(name="opsum", bufs=4, space="PSUM") as ops,
            tc.tile_pool(name="ostage", bufs=4) as ost,
        ):
            for st in range(ST):
                for ncol in range(4):
                    psum = ops.tile([128, 512], F32)
                    for hh in range(HPC):
                        nc.tensor.matmul(
                            psum,
                            ct_full[:, hh, st * 128 : (st + 1) * 128],
                            wo_sb[:, hh, ncol * 512 : (ncol + 1) * 512],
                            start=(hh == 0),
                            stop=(hh == HPC - 1),
                        )
                    o_sb = ost.tile([128, 512], F32, tag="ostage")
                    nc.scalar.activation(
                        out=o_sb, in_=psum, func=mybir.ActivationFunctionType.Copy
                    )
                    nc.gpsimd.dma_start(
                        out=out[
                            st * 128 : (st + 1) * 128,
                            ncol * 512 : (ncol + 1) * 512,
                        ],
                        in_=o_sb,
                    )
        wop_cm.__exit__(None, None, None)
        ctfull_cm.__exit__(None, None, None)


_NC = None


def _get_nc():
    global _NC
    if _NC is None:
        _NC = _build_nc()
    return _NC


def _host_prep(input_sequences, Wq, bq, Wk, bk, Wv, bv, Wo, bo):
    """Build per-core input maps."""
    x = np.asarray(input_sequences, dtype=np.float32)

    in_maps = []
    for c in range(8):
        b, g = divmod(c, 2)
        sl = slice(g * DHG, (g + 1) * DHG)
        wq_c = np.ascontiguousarray(
            np.asarray(Wq[:, sl], dtype=np.float32)
            .reshape(KT, 128, HPC, 128).transpose(2, 1, 0, 3).reshape(DHG, D)
        ).astype(np.float16)
        wk_c = np.ascontiguousarray(
            np.asarray(Wk[:, sl], dtype=np.float32)
            .reshape(KT, 128, HPC, 128).transpose(2, 1, 0, 3).reshape(DHG, D)
        ).astype(np.float16)
        wv_c = np.ascontiguousarray(Wv[:, sl]).astype(np.float16)
        wo_c = np.ascontiguousarray(Wo[sl, :]).astype(np.float16)
        in_maps.append({
            "xT": np.ascontiguousarray(x[b].T).astype(np.float16),
            "wq": wq_c,
            "wk": wk_c,
            "wv": wv_c,
            "wo": wo_c,
            "bqT": np.ascontiguousarray(
                np.asarray(bq[sl], dtype=np.float32).reshape(HPC, 128).T
            ),
            "bkT": np.ascontiguousarray(
                np.asarray(bk[sl], dtype=np.float32).reshape(HPC, 128).T
            ),
            "bvb": np.ascontiguousarray(
                np.broadcast_to(np.asarray(bv[sl], dtype=np.float32), (128, DHG))
            ),
        })
    return in_maps


def kernel(input_sequences, Wq, bq, Wk, bk, Wv, bv, Wo, bo, _trace=False):
    nc = _get_nc()
    in_maps = _host_prep(input_sequences, Wq, bq, Wk, bk, Wv, bv, Wo, bo)
    res = run_bass_kernel_spmd(nc, in_maps, list(range(8)), trace=_trace)
    bo32 = np.asarray(bo, dtype=np.float32)
    out = np.empty((B, S, D), dtype=np.float32)
    for b in range(B):
        out[b] = res.results[2 * b]["out"] + res.results[2 * b + 1]["out"] + bo32
    if _trace:
        kernel.last_exec_time_ns = res.exec_time_ns
    return out
